# revision 32
# baseline (speedup 1.0000x reference)
"""Multi-head self-attention with RoPE + causal mask on 8 Trainium2 NeuronCores.

Sharding: batch x head hybrid. Core c owns batch c//2 and head-half c%2
(8 of the 16 heads, a 512-wide slice of the QKV output dim / Wo input dim).
Each core computes a partial output out_c = O_c @ Wo_c^T of one batch; the
host sums the 2 partials per batch (the Wo row-split all-reduce done
host-side at gather time).

All matmul operands are bf16 (fp32 PSUM accumulation), which enables the
PE FastWeightLoad path and halves DMA/SBUF/DVE traffic vs fp32r. Per-core:
  - Q^T/K^T produced in [dims, S] layout (4 chunks of 128 partitions, 2
    heads per chunk); the RoPE de-interleave (even dims first, odd second,
    per head) is folded into Wq/Wk rows on the host. RoPE: DVE raw drain,
    PE permutation matmul for the +-32-partition swap, then cos/sin
    multiply-add on DVE with a sign-folded sin table.
  - V is produced directly in [seq, dims] layout (stationary = x tile,
    moving = Wv^T) -- no PE transposes. Stored per 128-seq chunk as
    [128, 8 heads, 65]: 64 dims + a ones column so the softmax
    denominators fall out of the P.V matmul for free.
  - ScoresT[sk, sq] = K^T.T @ Q^T per head; head pairs share one PSUM
    tile and run concurrently on the PE as (0,0)/(64,0) row tiles (K=64).
    exp on ScalarE from PSUM (1/8 scale folded; inputs bounded so no max
    subtraction). Causal masking multiplies only the 128x128 diagonal
    triangles.
  - The PE engine queue is strictly in-order, and the attention phase is
    ACT(exp)-bound, so emission order IS the schedule: P.V matmuls are
    emitted one chunk late (so they never head-of-line-block on their exp)
    and the NEXT rep's Q/K projection groups are interleaved between head
    pairs to fill the PE during the exp-bound stretches. V projections and
    the deferred last output-projection tile run at the top of the next
    rep. Output projections are emitted one sq tile late.
"""

import sys

sys.path.insert(0, "/opt/trn_rl_repo")

import numpy as np
from contextlib import ExitStack

import concourse.bass as bass
import concourse.tile as tile
from concourse import bacc, mybir
from concourse.bass_utils import run_bass_kernel_spmd

F32 = mybir.dt.float32
BF16 = mybir.dt.bfloat16

# problem constants (hardcoded per harness contract)
B = 4
S = 2048
D = 1024
NUM_HEADS = 16
DK = 64
THETA = 10000.0
NCORES = 8
HPC = 8  # heads per core
BLK = HPC * DK  # 512-wide per-core head-dim block
NDC = BLK // 128  # 4 dim chunks of 128
P = 128
SQT = 512  # sq tile width
NKC = D // P  # 8 contraction chunks for the projections


def build_program(b=B, s=S, reps=1):
    """Build the (SPMD-shared) per-core Bass program.

    reps>1 repeats the whole computation (for marginal-cost timing)."""
    nc = bacc.Bacc("TRN2", target_bir_lowering=False, debug=False)

    n_sqt = s // SQT  # sq tiles
    n_skc = s // P  # seq chunks of 128

    # ---- DRAM I/O ----
    xT = nc.dram_tensor("xT", [NKC, P, s], BF16, kind="ExternalInput").ap()
    wqT = nc.dram_tensor("wqT", [NKC, P, BLK], BF16, kind="ExternalInput").ap()
    wkT = nc.dram_tensor("wkT", [NKC, P, BLK], BF16, kind="ExternalInput").ap()
    wvT = nc.dram_tensor("wvT", [NKC, P, BLK], BF16, kind="ExternalInput").ap()
    woT = nc.dram_tensor("woT", [NDC, P, D], BF16, kind="ExternalInput").ap()
    costab = nc.dram_tensor("costab", [P, s], BF16, kind="ExternalInput").ap()
    sintab = nc.dram_tensor("sintab", [P, s], BF16, kind="ExternalInput").ap()
    pmswap = nc.dram_tensor("pmswap", [P, P], BF16, kind="ExternalInput").ap()
    causal = nc.dram_tensor("causal", [P, P], BF16, kind="ExternalInput").ap()
    out = nc.dram_tensor("out", [s, D], BF16, kind="ExternalOutput").ap()

    with tile.TileContext(nc) as tc, ExitStack() as ctx:
        consts = ctx.enter_context(tc.tile_pool(name="consts", bufs=1))
        # xt tiles stay alive from the (interleaved, early) Q/K groups until
        # the V projections at the top of the owning rep: ring of 4 covers
        # all four t-phases of a rep.
        xpool = ctx.enter_context(tc.tile_pool(name="xpool", bufs=4))
        big = ctx.enter_context(tc.tile_pool(name="big", bufs=1))
        # qT/kT double-buffered: the next rep's Q/K groups (written during
        # this rep's attention) must not clobber the tiles attention reads.
        qkp = ctx.enter_context(tc.tile_pool(name="qkp", bufs=2))
        work = ctx.enter_context(tc.tile_pool(name="work", bufs=2))
        expp = ctx.enter_context(tc.tile_pool(name="expp", bufs=3))
        psum = ctx.enter_context(tc.tile_pool(name="psum", bufs=2, space="PSUM"))
        opsum = ctx.enter_context(tc.tile_pool(name="opsum", bufs=1, space="PSUM"))

        # ---- constants resident in SBUF ----
        w_sb = {}
        for name, ap in (("wq", wqT), ("wk", wkT), ("wv", wvT)):
            t = consts.tile([P, NKC, BLK], BF16, tag=f"w_{name}")
            for kc in range(NKC):
                nc.sync.dma_start(t[:, kc], ap[kc])
            w_sb[name] = t
        wo_sb = consts.tile([P, NDC, D], BF16, tag="wo")
        for dc in range(NDC):
            nc.sync.dma_start(wo_sb[:, dc], woT[dc])
        cos_sb = consts.tile([P, s], BF16, tag="cos")
        nc.sync.dma_start(cos_sb[:], costab)
        sin_sb = consts.tile([P, s], BF16, tag="sin")
        nc.sync.dma_start(sin_sb[:], sintab)
        pm_sb = consts.tile([P, P], BF16, tag="pm")
        nc.sync.dma_start(pm_sb[:], pmswap)
        ca_sb = consts.tile([P, P], BF16, tag="causal")
        nc.sync.dma_start(ca_sb[:], causal)

        class RepState:
            def __init__(self):
                self.qT = qkp.tile([P, NDC, s], BF16, tag="qT", name="qT")
                self.kT = qkp.tile([P, NDC, s], BF16, tag="kT", name="kT")
                self.oT = None  # allocated at attention time
                self.xts = {}  # t_i -> list of 8 xt tiles (shared Q/K/V)
                self.v_sb = None

        def get_xt(st, t_i):
            if t_i not in st.xts:
                w = slice(t_i * SQT, (t_i + 1) * SQT)
                xt = []
                for kc in range(NKC):
                    xtc = xpool.tile([P, SQT], BF16, tag=f"xt{kc}", name=f"xt{kc}")
                    nc.sync.dma_start(xtc[:], xT[kc, :, w])
                    xt.append(xtc)
                st.xts[t_i] = xt
            return st.xts[t_i]

        def emit_qk_group(st, t_i, name, dc):
            """One Q or K projection group (8 matmuls) + fused RoPE drain."""
            w = slice(t_i * SQT, (t_i + 1) * SQT)
            xt = get_xt(st, t_i)
            dst = st.qT if name == "wq" else st.kT
            ps = psum.tile([P, SQT], F32, tag="mm", name="ps")
            for kc in range(NKC):
                nc.tensor.matmul(
                    ps[:],
                    w_sb[name][:, kc, dc * P : (dc + 1) * P],
                    xt[kc][:],
                    start=(kc == 0),
                    stop=(kc == NKC - 1),
                )
            # RoPE: rot = cos*q + sinsigma*swap(q)
            raw = work.tile([P, SQT], BF16, tag="raw")
            nc.vector.tensor_copy(raw[:], ps[:])
            ps_sw = psum.tile([P, SQT], F32, tag="mm", name="ps_sw")
            nc.tensor.matmul(ps_sw[:], pm_sb[:], raw[:], start=True, stop=True)
            tco = work.tile([P, SQT], BF16, tag="tco")
            nc.vector.tensor_tensor(
                tco[:], raw[:], cos_sb[:, w], mybir.AluOpType.mult
            )
            tsi = work.tile([P, SQT], BF16, tag="tsi")
            nc.vector.tensor_tensor(
                tsi[:], ps_sw[:], sin_sb[:, w], mybir.AluOpType.mult
            )
            nc.vector.tensor_tensor(
                dst[:, dc, w], tco[:], tsi[:], mybir.AluOpType.add
            )

        def emit_v(st):
            """All V projections (direct [seq, dims] layout + ones column)."""
            st.v_sb = [
                big.tile([P, HPC, DK + 1], BF16, tag=f"v{sc}", name=f"v{sc}")
                for sc in range(n_skc)
            ]
            for t_i in range(n_sqt):
                xt = get_xt(st, t_i)
                for sc8 in range(SQT // P):
                    sc = t_i * (SQT // P) + sc8
                    ps_v = psum.tile([P, SQT], F32, tag="mm", name="ps_v")
                    for kc in range(NKC):
                        nc.tensor.matmul(
                            ps_v[:],
                            xt[kc][:, sc8 * P : (sc8 + 1) * P],
                            w_sb["wv"][:, kc],
                            start=(kc == 0),
                            stop=(kc == NKC - 1),
                        )
                    vt = st.v_sb[sc]
                    nc.gpsimd.memset(vt[:, :, DK : DK + 1], 1.0)
                    nc.scalar.copy(vt[:, :, 0:DK], ps_v[:])

        def emit_proj(oT_src, sqt):
            # output projection for sq tile `sqt` (emitted one tile late so
            # the PE queue never stalls on the norm chain)
            for st_c in range(sqt * (SQT // P), (sqt + 1) * (SQT // P)):
                ob = work.tile([P, D], BF16, tag="ob")
                for nt in range(D // SQT):
                    ps_p = psum.tile([P, SQT], F32, tag="mm", name="ps_p")
                    for dc in range(NDC):
                        nc.tensor.matmul(
                            ps_p[:],
                            oT_src[:, dc, st_c * P : (st_c + 1) * P],
                            wo_sb[:, dc, nt * SQT : (nt + 1) * SQT],
                            start=(dc == 0),
                            stop=(dc == NDC - 1),
                        )
                    nc.vector.tensor_copy(ob[:, nt * SQT : (nt + 1) * SQT], ps_p[:])
                nc.sync.dma_start(out[st_c * P : (st_c + 1) * P, :], ob[:])

        def emit_attention(st, filler):
            """Attention for rep `st`; pops filler closures (next rep's Q/K
            groups) evenly spaced INSIDE the pass loop, between scores(k)
            and the lagged PV(k-1), so the in-order PE queue never
            head-of-line-blocks on an exp during the ACT-bound stretches."""
            st.oT = big.tile([P, NDC, s], BF16, tag="oT", name="oT")
            qT, kT, oT, v_sb = st.qT, st.kT, st.oT, st.v_sb
            n_pass = sum(4 * (sqt + 1) for sqt in range(n_sqt))  # 160
            stride = max(1, n_pass // len(filler)) if filler else 0
            pass_i = 0
            for sqt in range(n_sqt):
                sq0 = sqt * SQT
                nsk = (sq0 + SQT) // P
                for hp in range(HPC // 2):
                    po = opsum.tile([P, 2, SQT], F32, tag="po", name="po")
                    pv_pending = None

                    def emit_pv(skc, off):
                        for hi in range(2):
                            nc.tensor.matmul(
                                po[0 : DK + 1, hi, off:SQT],
                                v_sb[skc][:, 2 * hp + hi],
                                et_by_skc[skc][:, hi, off:SQT],
                                start=(skc == 0),
                                stop=(skc == nsk - 1),
                            )

                    et_by_skc = {}
                    for skc in range(nsk):
                        off = max(0, skc * P - sq0)
                        ps_s = psum.tile([P, 2, SQT], F32, tag="score", name="ps_s")
                        for hi in range(2):
                            p0 = hi * DK
                            nc.tensor.matmul(
                                ps_s[:, hi, off:SQT],
                                kT[p0 : p0 + DK, hp, skc * P : (skc + 1) * P],
                                qT[p0 : p0 + DK, hp, sq0 + off : sq0 + SQT],
                                start=True,
                                stop=True,
                            )
                        pass_i += 1
                        if filler and stride and pass_i % stride == 0:
                            filler.pop(0)()
                        if pv_pending is not None:
                            emit_pv(*pv_pending)
                        et = expp.tile([P, 2, SQT], BF16, tag="exp")
                        et_by_skc[skc] = et
                        nc.scalar.activation(
                            et[:, :, off:SQT],
                            ps_s[:, :, off:SQT],
                            mybir.ActivationFunctionType.Exp,
                            scale=float(1.0 / np.sqrt(DK)),
                        )
                        if skc * P >= sq0:  # diagonal chunk: mask invalid region
                            nc.vector.tensor_tensor(
                                et[:, :, off : off + P],
                                et[:, :, off : off + P],
                                ca_sb[:, None, :].to_broadcast([P, 2, P]),
                                mybir.AluOpType.mult,
                            )
                        pv_pending = (skc, off)
                    emit_pv(*pv_pending)

                    # normalize: oT[dims, sq] = po[0:64, sq] / po[64, sq].
                    # Single-op drain so the po bank frees ASAP.
                    osc = work.tile([DK + 1, 2, SQT], BF16, tag="osc", name="osc")
                    nc.vector.tensor_copy(osc[:], po[0 : DK + 1])
                    for hi in range(2):
                        rec = work.tile([1, SQT], BF16, tag="rec")
                        with nc.allow_low_precision(reason="softmax recip"):
                            nc.vector.reciprocal(rec[:], osc[DK : DK + 1, hi])
                        rec_bc = work.tile([DK, SQT], BF16, tag="rec_bc")
                        nc.gpsimd.partition_broadcast(rec_bc[:], rec[:])
                        nc.vector.tensor_tensor(
                            oT[hi * DK : (hi + 1) * DK, hp, sq0 : sq0 + SQT],
                            osc[0:DK, hi],
                            rec_bc[:],
                            mybir.AluOpType.mult,
                        )
                if sqt >= 1:
                    emit_proj(oT, sqt - 1)
            while filler:
                filler.pop(0)()

        def qk_filler(st):
            return [
                (lambda t_i=t_i, name=name, dc=dc: emit_qk_group(st, t_i, name, dc))
                for t_i in range(n_sqt)
                for name in ("wq", "wk")
                for dc in range(NDC)
            ]

        cur = RepState()
        for u in qk_filler(cur):  # rep 0: no previous attention to hide in
            u()
        pending_proj = None
        for r in range(reps):
            if pending_proj is not None:
                emit_proj(*pending_proj)  # before oT/v rings recycle
                pending_proj = None
            emit_v(cur)
            nxt = RepState() if r + 1 < reps else None
            emit_attention(cur, qk_filler(nxt) if nxt else [])
            pending_proj = (cur.oT, n_sqt - 1)
            cur = nxt

        if pending_proj is not None:
            emit_proj(*pending_proj)

    nc.compile()
    return nc


# ---------------- host side ----------------

_ROPE_PERM = None


def _rope_perm():
    """Per-head de-interleave: even dims first, then odd dims."""
    global _ROPE_PERM
    if _ROPE_PERM is None:
        p = []
        for h in range(HPC):
            base = h * DK
            p += [base + 2 * k for k in range(DK // 2)]
            p += [base + 2 * k + 1 for k in range(DK // 2)]
        _ROPE_PERM = np.array(p)
    return _ROPE_PERM


def _bf16():
    import ml_dtypes

    return ml_dtypes.bfloat16


def _host_tables(token_positions, s):
    pos = np.asarray(token_positions).astype(np.float64)
    freqs = THETA ** (-np.arange(0, DK, 2, dtype=np.float64) / DK)  # [32]
    ang = pos[None, :] * freqs[:, None]  # [32, s]
    cos32 = np.cos(ang)
    sin32 = np.sin(ang)
    # layout [128, s]: per head block of 64: [cos32 (x1 half); cos32 (x2 half)]
    cos_t = np.empty((P, s), np.float32)
    sin_t = np.empty((P, s), np.float32)
    for h in range(2):  # 2 heads per 128-partition chunk
        b0 = h * DK
        cos_t[b0 : b0 + 32] = cos32
        cos_t[b0 + 32 : b0 + 64] = cos32
        sin_t[b0 : b0 + 32] = -sin32  # x1 half: -sin * x2
        sin_t[b0 + 32 : b0 + 64] = sin32  # x2 half: +sin * x1
    return cos_t, sin_t


_NC_CACHE = {}

# test harness hooks (off by default; harness calls kernel() directly)
TRACE = False
LAST = {}


def _get_program(b, s, reps=1):
    key = (b, s, reps)
    if key not in _NC_CACHE:
        _NC_CACHE[key] = build_program(b, s, reps)
    return _NC_CACHE[key]


def prepare_in_maps(x, Wq, Wk, Wv, Wo, token_positions):
    bf16 = _bf16()
    x = np.asarray(x, dtype=np.float32)
    Wq = np.asarray(Wq, dtype=np.float32)
    Wk = np.asarray(Wk, dtype=np.float32)
    Wv = np.asarray(Wv, dtype=np.float32)
    Wo = np.asarray(Wo, dtype=np.float32)
    b, s, _ = x.shape

    # [b, kc, p, s] transposed view of x
    xT = np.ascontiguousarray(x.transpose(0, 2, 1)).astype(bf16).reshape(
        b, NKC, P, s
    )
    cos_t, sin_t = _host_tables(token_positions, s)
    cos_t = cos_t.astype(bf16)
    sin_t = sin_t.astype(bf16)
    causal = np.triu(np.ones((P, P), np.float32)).astype(bf16)  # keep p <= f
    # swap permutation matrix (symmetric): swap(j) = j+-32 within each 64-block
    pm = np.zeros((P, P), np.float32)
    for h in range(2):
        b0 = h * DK
        for k in range(32):
            pm[b0 + k + 32, b0 + k] = 1.0
            pm[b0 + k, b0 + k + 32] = 1.0
    pm = pm.astype(bf16)

    perm = _rope_perm()
    in_maps = []
    for c in range(NCORES):
        bi, hh = c // 2, c % 2
        rows = slice(hh * BLK, (hh + 1) * BLK)
        wq_c = Wq[rows][perm]  # [512, D] rope-permuted rows
        wk_c = Wk[rows][perm]
        wv_c = Wv[rows]
        in_maps.append(
            {
                "xT": xT[bi],
                "wqT": np.ascontiguousarray(wq_c.T).astype(bf16).reshape(NKC, P, BLK),
                "wkT": np.ascontiguousarray(wk_c.T).astype(bf16).reshape(NKC, P, BLK),
                "wvT": np.ascontiguousarray(wv_c.T).astype(bf16).reshape(NKC, P, BLK),
                "woT": np.ascontiguousarray(Wo[:, rows].T)
                .astype(bf16)
                .reshape(NDC, P, D),
                "costab": cos_t,
                "sintab": sin_t,
                "pmswap": pm,
                "causal": causal,
            }
        )

    return in_maps


def kernel(x, Wq, Wk, Wv, Wo, token_positions):
    b, s, _ = np.asarray(x).shape
    nc = _get_program(b, s)
    in_maps = prepare_in_maps(x, Wq, Wk, Wv, Wo, token_positions)
    res = run_bass_kernel_spmd(
        nc, in_maps, core_ids=list(range(NCORES)), trace=TRACE
    )
    LAST["exec_time_ns"] = res.exec_time_ns
    LAST["profile_json"] = res.profile_json
    out = np.empty((b, s, D), np.float32)
    for bi in range(b):
        out[bi] = res.results[2 * bi]["out"].astype(np.float32) + res.results[
            2 * bi + 1
        ]["out"].astype(np.float32)
    return out


# revision 35
# speedup vs baseline: 1.1808x; 1.1808x over previous
"""Multi-head self-attention with RoPE + causal mask on 8 Trainium2 NeuronCores.

Sharding: batch x head hybrid. Core c owns batch c//2 and head-half c%2
(8 of the 16 heads, a 512-wide slice of the QKV output dim / Wo input dim).
Each core computes a partial output out_c = O_c @ Wo_c^T of one batch; the
host sums the 2 partials per batch (the Wo row-split all-reduce done
host-side at gather time).

All matmul operands are bf16 (fp32 PSUM accumulation), which enables the
PE FastWeightLoad path and halves DMA/SBUF/DVE traffic vs fp32r. Per-core:
  - Q^T/K^T produced in [dims, S] layout (4 chunks of 128 partitions, 2
    heads per chunk); the RoPE de-interleave (even dims first, odd second,
    per head) is folded into Wq/Wk rows on the host. RoPE: DVE raw drain,
    PE permutation matmul for the +-32-partition swap, then cos/sin
    multiply-add on DVE with a sign-folded sin table.
  - V is produced directly in [seq, dims] layout (stationary = x tile,
    moving = Wv^T) -- no PE transposes. Stored per 128-seq chunk as
    [128, 8 heads, 65]: 64 dims + a ones column so the softmax
    denominators fall out of the P.V matmul for free.
  - ScoresT[sk, sq] = K^T.T @ Q^T per head; head pairs share one PSUM
    tile and run concurrently on the PE as (0,0)/(64,0) row tiles (K=64).
    exp on ScalarE from PSUM (1/8 scale folded; inputs bounded so no max
    subtraction). Causal masking multiplies only the 128x128 diagonal
    triangles.
  - The PE engine queue is strictly in-order, and the attention phase is
    ACT(exp)-bound, so emission order IS the schedule: P.V matmuls are
    emitted one chunk late (so they never head-of-line-block on their exp)
    and the NEXT rep's Q/K projection groups are interleaved between head
    pairs to fill the PE during the exp-bound stretches. V projections and
    the deferred last output-projection tile run at the top of the next
    rep. Output projections are emitted one sq tile late.
"""

import sys

sys.path.insert(0, "/opt/trn_rl_repo")

import numpy as np
from contextlib import ExitStack

import concourse.bass as bass
import concourse.tile as tile
from concourse import bacc, mybir
from concourse.bass_utils import run_bass_kernel_spmd

F32 = mybir.dt.float32
BF16 = mybir.dt.bfloat16

# problem constants (hardcoded per harness contract)
B = 4
S = 2048
D = 1024
NUM_HEADS = 16
DK = 64
THETA = 10000.0
NCORES = 8
HPC = 8  # heads per core
BLK = HPC * DK  # 512-wide per-core head-dim block
NDC = BLK // 128  # 4 dim chunks of 128
P = 128
SQT = 512  # sq tile width
NKC = D // P  # 8 contraction chunks for the projections


def build_program(b=B, s=S, reps=1):
    """Build the (SPMD-shared) per-core Bass program.

    reps>1 repeats the whole computation (for marginal-cost timing)."""
    nc = bacc.Bacc("TRN2", target_bir_lowering=False, debug=False)

    n_sqt = s // SQT  # sq tiles
    n_skc = s // P  # seq chunks of 128

    # ---- DRAM I/O ----
    xT = nc.dram_tensor("xT", [NKC, P, s], BF16, kind="ExternalInput").ap()
    wqT = nc.dram_tensor("wqT", [NKC, P, BLK], BF16, kind="ExternalInput").ap()
    wkT = nc.dram_tensor("wkT", [NKC, P, BLK], BF16, kind="ExternalInput").ap()
    wvT = nc.dram_tensor("wvT", [NKC, P, BLK], BF16, kind="ExternalInput").ap()
    woT = nc.dram_tensor("woT", [NDC, P, D], BF16, kind="ExternalInput").ap()
    costab = nc.dram_tensor("costab", [P, s], BF16, kind="ExternalInput").ap()
    sintab = nc.dram_tensor("sintab", [P, s], BF16, kind="ExternalInput").ap()
    pmswap = nc.dram_tensor("pmswap", [P, P], BF16, kind="ExternalInput").ap()
    causal = nc.dram_tensor("causal", [P, P], BF16, kind="ExternalInput").ap()
    out = nc.dram_tensor("out", [s, D], BF16, kind="ExternalOutput").ap()

    with tile.TileContext(nc) as tc, ExitStack() as ctx:
        consts = ctx.enter_context(tc.tile_pool(name="consts", bufs=1))
        # xt tiles stay alive from the (interleaved, early) Q/K groups until
        # the V projections at the top of the owning rep: ring of 4 covers
        # all four t-phases of a rep.
        xpool = ctx.enter_context(tc.tile_pool(name="xpool", bufs=4))
        big = ctx.enter_context(tc.tile_pool(name="big", bufs=1))
        # qT/kT double-buffered: the next rep's Q/K groups (written during
        # this rep's attention) must not clobber the tiles attention reads.
        qkp = ctx.enter_context(tc.tile_pool(name="qkp", bufs=2))
        work = ctx.enter_context(tc.tile_pool(name="work", bufs=2))
        expp = ctx.enter_context(tc.tile_pool(name="expp", bufs=4))
        psum = ctx.enter_context(tc.tile_pool(name="psum", bufs=2, space="PSUM"))
        opsum = ctx.enter_context(tc.tile_pool(name="opsum", bufs=1, space="PSUM"))

        # ---- constants resident in SBUF ----
        w_sb = {}
        for name, ap in (("wq", wqT), ("wk", wkT), ("wv", wvT)):
            t = consts.tile([P, NKC, BLK], BF16, tag=f"w_{name}")
            for kc in range(NKC):
                nc.sync.dma_start(t[:, kc], ap[kc])
            w_sb[name] = t
        wo_sb = consts.tile([P, NDC, D], BF16, tag="wo")
        for dc in range(NDC):
            nc.sync.dma_start(wo_sb[:, dc], woT[dc])
        cos_sb = consts.tile([P, s], BF16, tag="cos")
        nc.sync.dma_start(cos_sb[:], costab)
        sin_sb = consts.tile([P, s], BF16, tag="sin")
        nc.sync.dma_start(sin_sb[:], sintab)
        pm_sb = consts.tile([P, P], BF16, tag="pm")
        nc.sync.dma_start(pm_sb[:], pmswap)
        ca_sb = consts.tile([P, P], BF16, tag="causal")
        nc.sync.dma_start(ca_sb[:], causal)

        class RepState:
            def __init__(self):
                self.qT = qkp.tile([P, NDC, s], BF16, tag="qT", name="qT")
                self.kT = qkp.tile([P, NDC, s], BF16, tag="kT", name="kT")
                self.oT = None  # allocated at attention time
                self.xts = {}  # t_i -> list of 8 xt tiles (shared Q/K/V)
                self.v_sb = None

        def get_xt(st, t_i):
            if t_i not in st.xts:
                w = slice(t_i * SQT, (t_i + 1) * SQT)
                xt = []
                for kc in range(NKC):
                    xtc = xpool.tile([P, SQT], BF16, tag=f"xt{kc}", name=f"xt{kc}")
                    nc.sync.dma_start(xtc[:], xT[kc, :, w])
                    xt.append(xtc)
                st.xts[t_i] = xt
            return st.xts[t_i]

        def emit_qk_group(st, t_i, name, dc):
            """One Q or K projection group (8 matmuls) + fused RoPE drain."""
            w = slice(t_i * SQT, (t_i + 1) * SQT)
            xt = get_xt(st, t_i)
            dst = st.qT if name == "wq" else st.kT
            ps = psum.tile([P, SQT], F32, tag="mm", name="ps")
            for kc in range(NKC):
                nc.tensor.matmul(
                    ps[:],
                    w_sb[name][:, kc, dc * P : (dc + 1) * P],
                    xt[kc][:],
                    start=(kc == 0),
                    stop=(kc == NKC - 1),
                )
            # RoPE: rot = cos*q + sinsigma*swap(q)
            raw = work.tile([P, SQT], BF16, tag="raw")
            nc.vector.tensor_copy(raw[:], ps[:])
            ps_sw = psum.tile([P, SQT], F32, tag="mm", name="ps_sw")
            nc.tensor.matmul(ps_sw[:], pm_sb[:], raw[:], start=True, stop=True)
            tco = work.tile([P, SQT], BF16, tag="tco")
            nc.vector.tensor_tensor(
                tco[:], raw[:], cos_sb[:, w], mybir.AluOpType.mult
            )
            tsi = work.tile([P, SQT], BF16, tag="tsi")
            nc.vector.tensor_tensor(
                tsi[:], ps_sw[:], sin_sb[:, w], mybir.AluOpType.mult
            )
            nc.vector.tensor_tensor(
                dst[:, dc, w], tco[:], tsi[:], mybir.AluOpType.add
            )

        def emit_v(st):
            """All V projections (direct [seq, dims] layout + ones column)."""
            st.v_sb = [
                big.tile([P, HPC, DK + 1], BF16, tag=f"v{sc}", name=f"v{sc}")
                for sc in range(n_skc)
            ]
            for t_i in range(n_sqt):
                xt = get_xt(st, t_i)
                for sc8 in range(SQT // P):
                    sc = t_i * (SQT // P) + sc8
                    ps_v = psum.tile([P, SQT], F32, tag="mm", name="ps_v")
                    for kc in range(NKC):
                        nc.tensor.matmul(
                            ps_v[:],
                            xt[kc][:, sc8 * P : (sc8 + 1) * P],
                            w_sb["wv"][:, kc],
                            start=(kc == 0),
                            stop=(kc == NKC - 1),
                        )
                    vt = st.v_sb[sc]
                    nc.gpsimd.memset(vt[:, :, DK : DK + 1], 1.0)
                    nc.scalar.copy(vt[:, :, 0:DK], ps_v[:])

        def emit_proj(oT_src, sqt):
            # output projection for sq tile `sqt` (emitted one tile late so
            # the PE queue never stalls on the norm chain)
            for st_c in range(sqt * (SQT // P), (sqt + 1) * (SQT // P)):
                ob = work.tile([P, D], BF16, tag="ob")
                for nt in range(D // SQT):
                    ps_p = psum.tile([P, SQT], F32, tag="mm", name="ps_p")
                    for dc in range(NDC):
                        nc.tensor.matmul(
                            ps_p[:],
                            oT_src[:, dc, st_c * P : (st_c + 1) * P],
                            wo_sb[:, dc, nt * SQT : (nt + 1) * SQT],
                            start=(dc == 0),
                            stop=(dc == NDC - 1),
                        )
                    nc.vector.tensor_copy(ob[:, nt * SQT : (nt + 1) * SQT], ps_p[:])
                nc.sync.dma_start(out[st_c * P : (st_c + 1) * P, :], ob[:])

        def emit_attention(st, filler):
            """Attention for rep `st`. P.V matmuls are emitted TWO passes
            late so each one has ~2 score-matmuls of PE shadow to cover its
            exp's latency (the PE queue is strictly in-order). Next-rep Q/K
            filler groups are popped at head-pair boundaries, absorbing the
            norm-chain and po-ring latency there."""
            st.oT = big.tile([P, NDC, s], BF16, tag="oT", name="oT")
            qT, kT, oT, v_sb = st.qT, st.kT, st.oT, st.v_sb
            for sqt in range(n_sqt):
                sq0 = sqt * SQT
                nsk = (sq0 + SQT) // P
                for hp in range(HPC // 2):
                    po = opsum.tile([P, 2, SQT], F32, tag="po", name="po")
                    pv_queue = []

                    def emit_pv(skc, off):
                        for hi in range(2):
                            nc.tensor.matmul(
                                po[0 : DK + 1, hi, off:SQT],
                                v_sb[skc][:, 2 * hp + hi],
                                et_by_skc[skc][:, hi, off:SQT],
                                start=(skc == 0),
                                stop=(skc == nsk - 1),
                            )

                    et_by_skc = {}
                    for skc in range(nsk):
                        off = max(0, skc * P - sq0)
                        ps_s = psum.tile([P, 2, SQT], F32, tag="score", name="ps_s")
                        for hi in range(2):
                            p0 = hi * DK
                            nc.tensor.matmul(
                                ps_s[:, hi, off:SQT],
                                kT[p0 : p0 + DK, hp, skc * P : (skc + 1) * P],
                                qT[p0 : p0 + DK, hp, sq0 + off : sq0 + SQT],
                                start=True,
                                stop=True,
                            )
                        if len(pv_queue) >= 2:
                            emit_pv(*pv_queue.pop(0))
                        et = expp.tile([P, 2, SQT], BF16, tag="exp")
                        et_by_skc[skc] = et
                        nc.scalar.activation(
                            et[:, :, off:SQT],
                            ps_s[:, :, off:SQT],
                            mybir.ActivationFunctionType.Exp,
                            scale=float(1.0 / np.sqrt(DK)),
                        )
                        if skc * P >= sq0:  # diagonal chunk: mask invalid region
                            nc.vector.tensor_tensor(
                                et[:, :, off : off + P],
                                et[:, :, off : off + P],
                                ca_sb[:, None, :].to_broadcast([P, 2, P]),
                                mybir.AluOpType.mult,
                            )
                        pv_queue.append((skc, off))
                    while pv_queue:
                        emit_pv(*pv_queue.pop(0))

                    # normalize: oT[dims, sq] = po[0:64, sq] / po[64, sq].
                    # Single-op drain so the po bank frees ASAP.
                    osc = work.tile([DK + 1, 2, SQT], BF16, tag="osc", name="osc")
                    nc.vector.tensor_copy(osc[:], po[0 : DK + 1])
                    for hi in range(2):
                        rec = work.tile([1, SQT], BF16, tag="rec")
                        with nc.allow_low_precision(reason="softmax recip"):
                            nc.vector.reciprocal(rec[:], osc[DK : DK + 1, hi])
                        rec_bc = work.tile([DK, SQT], BF16, tag="rec_bc")
                        nc.gpsimd.partition_broadcast(rec_bc[:], rec[:])
                        nc.vector.tensor_tensor(
                            oT[hi * DK : (hi + 1) * DK, hp, sq0 : sq0 + SQT],
                            osc[0:DK, hi],
                            rec_bc[:],
                            mybir.AluOpType.mult,
                        )
                    # keep the PE fed across the hp boundary (norm + po ring)
                    for _ in range(2):
                        if filler:
                            filler.pop(0)()
                if sqt >= 1:
                    emit_proj(oT, sqt - 1)
            while filler:
                filler.pop(0)()

        def qk_filler(st):
            return [
                (lambda t_i=t_i, name=name, dc=dc: emit_qk_group(st, t_i, name, dc))
                for t_i in range(n_sqt)
                for name in ("wq", "wk")
                for dc in range(NDC)
            ]

        cur = RepState()
        for u in qk_filler(cur):  # rep 0: no previous attention to hide in
            u()
        pending_proj = None
        for r in range(reps):
            if pending_proj is not None:
                emit_proj(*pending_proj)  # before oT/v rings recycle
                pending_proj = None
            emit_v(cur)
            nxt = RepState() if r + 1 < reps else None
            emit_attention(cur, qk_filler(nxt) if nxt else [])
            pending_proj = (cur.oT, n_sqt - 1)
            cur = nxt

        if pending_proj is not None:
            emit_proj(*pending_proj)

    nc.compile()
    return nc


# ---------------- host side ----------------

_ROPE_PERM = None


def _rope_perm():
    """Per-head de-interleave: even dims first, then odd dims."""
    global _ROPE_PERM
    if _ROPE_PERM is None:
        p = []
        for h in range(HPC):
            base = h * DK
            p += [base + 2 * k for k in range(DK // 2)]
            p += [base + 2 * k + 1 for k in range(DK // 2)]
        _ROPE_PERM = np.array(p)
    return _ROPE_PERM


def _bf16():
    import ml_dtypes

    return ml_dtypes.bfloat16


def _host_tables(token_positions, s):
    pos = np.asarray(token_positions).astype(np.float64)
    freqs = THETA ** (-np.arange(0, DK, 2, dtype=np.float64) / DK)  # [32]
    ang = pos[None, :] * freqs[:, None]  # [32, s]
    cos32 = np.cos(ang)
    sin32 = np.sin(ang)
    # layout [128, s]: per head block of 64: [cos32 (x1 half); cos32 (x2 half)]
    cos_t = np.empty((P, s), np.float32)
    sin_t = np.empty((P, s), np.float32)
    for h in range(2):  # 2 heads per 128-partition chunk
        b0 = h * DK
        cos_t[b0 : b0 + 32] = cos32
        cos_t[b0 + 32 : b0 + 64] = cos32
        sin_t[b0 : b0 + 32] = -sin32  # x1 half: -sin * x2
        sin_t[b0 + 32 : b0 + 64] = sin32  # x2 half: +sin * x1
    return cos_t, sin_t


_NC_CACHE = {}

# test harness hooks (off by default; harness calls kernel() directly)
TRACE = False
LAST = {}


def _get_program(b, s, reps=1):
    key = (b, s, reps)
    if key not in _NC_CACHE:
        _NC_CACHE[key] = build_program(b, s, reps)
    return _NC_CACHE[key]


def prepare_in_maps(x, Wq, Wk, Wv, Wo, token_positions):
    bf16 = _bf16()
    x = np.asarray(x, dtype=np.float32)
    Wq = np.asarray(Wq, dtype=np.float32)
    Wk = np.asarray(Wk, dtype=np.float32)
    Wv = np.asarray(Wv, dtype=np.float32)
    Wo = np.asarray(Wo, dtype=np.float32)
    b, s, _ = x.shape

    # [b, kc, p, s] transposed view of x
    xT = np.ascontiguousarray(x.transpose(0, 2, 1)).astype(bf16).reshape(
        b, NKC, P, s
    )
    cos_t, sin_t = _host_tables(token_positions, s)
    cos_t = cos_t.astype(bf16)
    sin_t = sin_t.astype(bf16)
    causal = np.triu(np.ones((P, P), np.float32)).astype(bf16)  # keep p <= f
    # swap permutation matrix (symmetric): swap(j) = j+-32 within each 64-block
    pm = np.zeros((P, P), np.float32)
    for h in range(2):
        b0 = h * DK
        for k in range(32):
            pm[b0 + k + 32, b0 + k] = 1.0
            pm[b0 + k, b0 + k + 32] = 1.0
    pm = pm.astype(bf16)

    perm = _rope_perm()
    in_maps = []
    for c in range(NCORES):
        bi, hh = c // 2, c % 2
        rows = slice(hh * BLK, (hh + 1) * BLK)
        wq_c = Wq[rows][perm]  # [512, D] rope-permuted rows
        wk_c = Wk[rows][perm]
        wv_c = Wv[rows]
        in_maps.append(
            {
                "xT": xT[bi],
                "wqT": np.ascontiguousarray(wq_c.T).astype(bf16).reshape(NKC, P, BLK),
                "wkT": np.ascontiguousarray(wk_c.T).astype(bf16).reshape(NKC, P, BLK),
                "wvT": np.ascontiguousarray(wv_c.T).astype(bf16).reshape(NKC, P, BLK),
                "woT": np.ascontiguousarray(Wo[:, rows].T)
                .astype(bf16)
                .reshape(NDC, P, D),
                "costab": cos_t,
                "sintab": sin_t,
                "pmswap": pm,
                "causal": causal,
            }
        )

    return in_maps


def kernel(x, Wq, Wk, Wv, Wo, token_positions):
    b, s, _ = np.asarray(x).shape
    nc = _get_program(b, s)
    in_maps = prepare_in_maps(x, Wq, Wk, Wv, Wo, token_positions)
    res = run_bass_kernel_spmd(
        nc, in_maps, core_ids=list(range(NCORES)), trace=TRACE
    )
    LAST["exec_time_ns"] = res.exec_time_ns
    LAST["profile_json"] = res.profile_json
    out = np.empty((b, s, D), np.float32)
    for bi in range(b):
        out[bi] = res.results[2 * bi]["out"].astype(np.float32) + res.results[
            2 * bi + 1
        ]["out"].astype(np.float32)
    return out


# revision 40
# speedup vs baseline: 1.1996x; 1.0159x over previous
"""Multi-head self-attention with RoPE + causal mask on 8 Trainium2 NeuronCores.

Sharding: batch x head hybrid. Core c owns batch c//2 and head-half c%2
(8 of the 16 heads, a 512-wide slice of the QKV output dim / Wo input dim).
Each core computes a partial output out_c = O_c @ Wo_c^T of one batch; the
host sums the 2 partials per batch (the Wo row-split all-reduce done
host-side at gather time).

All matmul operands are bf16 (fp32 PSUM accumulation), which enables the
PE FastWeightLoad path and halves DMA/SBUF/DVE traffic vs fp32r. Per-core:
  - Q^T/K^T produced in [dims, S] layout (4 chunks of 128 partitions, 2
    heads per chunk); the RoPE de-interleave (even dims first, odd second,
    per head) is folded into Wq/Wk rows on the host. RoPE: DVE raw drain,
    PE permutation matmul for the +-32-partition swap, then cos/sin
    multiply-add on DVE with a sign-folded sin table.
  - V is produced directly in [seq, dims] layout (stationary = x tile,
    moving = Wv^T) -- no PE transposes. Stored per 128-seq chunk as
    [128, 8 heads, 65]: 64 dims + a ones column so the softmax
    denominators fall out of the P.V matmul for free.
  - ScoresT[sk, sq] = K^T.T @ Q^T per head; head pairs share one PSUM
    tile and run concurrently on the PE as (0,0)/(64,0) row tiles (K=64).
    exp on ScalarE from PSUM (1/8 scale folded; inputs bounded so no max
    subtraction). Causal masking multiplies only the 128x128 diagonal
    triangles.
  - The PE engine queue is strictly in-order, and the attention phase is
    ACT(exp)-bound, so emission order IS the schedule: P.V matmuls are
    emitted one chunk late (so they never head-of-line-block on their exp)
    and the NEXT rep's Q/K projection groups are interleaved between head
    pairs to fill the PE during the exp-bound stretches. V projections and
    the deferred last output-projection tile run at the top of the next
    rep. Output projections are emitted one sq tile late.
"""

import sys

sys.path.insert(0, "/opt/trn_rl_repo")

import numpy as np
from contextlib import ExitStack

import concourse.bass as bass
import concourse.tile as tile
from concourse import bacc, mybir
from concourse.bass_utils import run_bass_kernel_spmd

F32 = mybir.dt.float32
BF16 = mybir.dt.bfloat16

# problem constants (hardcoded per harness contract)
B = 4
S = 2048
D = 1024
NUM_HEADS = 16
DK = 64
THETA = 10000.0
NCORES = 8
HPC = 8  # heads per core
BLK = HPC * DK  # 512-wide per-core head-dim block
NDC = BLK // 128  # 4 dim chunks of 128
P = 128
SQT = 512  # sq tile width
NKC = D // P  # 8 contraction chunks for the projections


def build_program(b=B, s=S, reps=1):
    """Build the (SPMD-shared) per-core Bass program.

    reps>1 repeats the whole computation (for marginal-cost timing)."""
    nc = bacc.Bacc("TRN2", target_bir_lowering=False, debug=False)

    n_sqt = s // SQT  # sq tiles
    n_skc = s // P  # seq chunks of 128

    # ---- DRAM I/O ----
    xT = nc.dram_tensor("xT", [NKC, P, s], BF16, kind="ExternalInput").ap()
    wqT = nc.dram_tensor("wqT", [NKC, P, BLK], BF16, kind="ExternalInput").ap()
    wkT = nc.dram_tensor("wkT", [NKC, P, BLK], BF16, kind="ExternalInput").ap()
    wvT = nc.dram_tensor("wvT", [NKC, P, BLK], BF16, kind="ExternalInput").ap()
    woT = nc.dram_tensor("woT", [NDC, P, D], BF16, kind="ExternalInput").ap()
    costab = nc.dram_tensor("costab", [P, s], BF16, kind="ExternalInput").ap()
    sintab = nc.dram_tensor("sintab", [P, s], BF16, kind="ExternalInput").ap()
    pmswap = nc.dram_tensor("pmswap", [P, P], BF16, kind="ExternalInput").ap()
    causal = nc.dram_tensor("causal", [P, P], BF16, kind="ExternalInput").ap()
    out = nc.dram_tensor("out", [s, D], BF16, kind="ExternalOutput").ap()

    with tile.TileContext(nc) as tc, ExitStack() as ctx:
        consts = ctx.enter_context(tc.tile_pool(name="consts", bufs=1))
        # xt tiles stay alive from the (interleaved, early) Q/K groups until
        # the V projections at the top of the owning rep: ring of 4 covers
        # all four t-phases of a rep.
        xpool = ctx.enter_context(tc.tile_pool(name="xpool", bufs=4))
        big = ctx.enter_context(tc.tile_pool(name="big", bufs=1))
        # qT/kT double-buffered: the next rep's Q/K groups (written during
        # this rep's attention) must not clobber the tiles attention reads.
        qkp = ctx.enter_context(tc.tile_pool(name="qkp", bufs=2))
        work = ctx.enter_context(tc.tile_pool(name="work", bufs=2))
        expp = ctx.enter_context(tc.tile_pool(name="expp", bufs=3))
        psum = ctx.enter_context(tc.tile_pool(name="psum", bufs=2, space="PSUM"))
        opsum = ctx.enter_context(tc.tile_pool(name="opsum", bufs=1, space="PSUM"))

        # ---- constants resident in SBUF ----
        w_sb = {}
        for name, ap in (("wq", wqT), ("wk", wkT), ("wv", wvT)):
            t = consts.tile([P, NKC, BLK], BF16, tag=f"w_{name}")
            for kc in range(NKC):
                nc.sync.dma_start(t[:, kc], ap[kc])
            w_sb[name] = t
        wo_sb = consts.tile([P, NDC, D], BF16, tag="wo")
        for dc in range(NDC):
            nc.sync.dma_start(wo_sb[:, dc], woT[dc])
        cos_sb = consts.tile([P, s], BF16, tag="cos")
        nc.sync.dma_start(cos_sb[:], costab)
        sin_sb = consts.tile([P, s], BF16, tag="sin")
        nc.sync.dma_start(sin_sb[:], sintab)
        pm_sb = consts.tile([P, P], BF16, tag="pm")
        nc.sync.dma_start(pm_sb[:], pmswap)
        ca_sb = consts.tile([P, P], BF16, tag="causal")
        nc.sync.dma_start(ca_sb[:], causal)

        class RepState:
            def __init__(self):
                self.qT = qkp.tile([P, NDC, s], BF16, tag="qT", name="qT")
                self.kT = qkp.tile([P, NDC, s], BF16, tag="kT", name="kT")
                self.oT = None  # allocated at attention time
                self.xts = {}  # t_i -> list of 8 xt tiles (shared Q/K/V)
                self.v_sb = None

        def get_xt(st, t_i):
            if t_i not in st.xts:
                w = slice(t_i * SQT, (t_i + 1) * SQT)
                xt = []
                for kc in range(NKC):
                    xtc = xpool.tile([P, SQT], BF16, tag=f"xt{kc}", name=f"xt{kc}")
                    nc.sync.dma_start(xtc[:], xT[kc, :, w])
                    xt.append(xtc)
                st.xts[t_i] = xt
            return st.xts[t_i]

        def emit_qk_group(st, t_i, name, dc):
            """One Q or K projection group (8 matmuls) + fused RoPE drain."""
            w = slice(t_i * SQT, (t_i + 1) * SQT)
            xt = get_xt(st, t_i)
            dst = st.qT if name == "wq" else st.kT
            ps = psum.tile([P, SQT], F32, tag="mm", name="ps")
            for kc in range(NKC):
                nc.tensor.matmul(
                    ps[:],
                    w_sb[name][:, kc, dc * P : (dc + 1) * P],
                    xt[kc][:],
                    start=(kc == 0),
                    stop=(kc == NKC - 1),
                )
            # RoPE: rot = cos*q + sinsigma*swap(q)
            raw = work.tile([P, SQT], BF16, tag="raw")
            nc.vector.tensor_copy(raw[:], ps[:])
            ps_sw = psum.tile([P, SQT], F32, tag="mm", name="ps_sw")
            nc.tensor.matmul(ps_sw[:], pm_sb[:], raw[:], start=True, stop=True)
            tco = work.tile([P, SQT], BF16, tag="tco")
            nc.vector.tensor_tensor(
                tco[:], raw[:], cos_sb[:, w], mybir.AluOpType.mult
            )
            tsi = work.tile([P, SQT], BF16, tag="tsi")
            nc.vector.tensor_tensor(
                tsi[:], ps_sw[:], sin_sb[:, w], mybir.AluOpType.mult
            )
            nc.vector.tensor_tensor(
                dst[:, dc, w], tco[:], tsi[:], mybir.AluOpType.add
            )

        def emit_v(st):
            """All V projections (direct [seq, dims] layout + ones column)."""
            st.v_sb = [
                big.tile([P, HPC, DK + 1], BF16, tag=f"v{sc}", name=f"v{sc}")
                for sc in range(n_skc)
            ]
            for t_i in range(n_sqt):
                xt = get_xt(st, t_i)
                for sc8 in range(SQT // P):
                    sc = t_i * (SQT // P) + sc8
                    ps_v = psum.tile([P, SQT], F32, tag="mm", name="ps_v")
                    for kc in range(NKC):
                        nc.tensor.matmul(
                            ps_v[:],
                            xt[kc][:, sc8 * P : (sc8 + 1) * P],
                            w_sb["wv"][:, kc],
                            start=(kc == 0),
                            stop=(kc == NKC - 1),
                        )
                    vt = st.v_sb[sc]
                    nc.gpsimd.memset(vt[:, :, DK : DK + 1], 1.0)
                    nc.scalar.copy(vt[:, :, 0:DK], ps_v[:])

        def emit_proj(oT_src, sqt):
            # output projection for sq tile `sqt` (emitted one tile late so
            # the PE queue never stalls on the norm chain)
            for st_c in range(sqt * (SQT // P), (sqt + 1) * (SQT // P)):
                ob = work.tile([P, D], BF16, tag="ob")
                for nt in range(D // SQT):
                    ps_p = psum.tile([P, SQT], F32, tag="mm", name="ps_p")
                    for dc in range(NDC):
                        nc.tensor.matmul(
                            ps_p[:],
                            oT_src[:, dc, st_c * P : (st_c + 1) * P],
                            wo_sb[:, dc, nt * SQT : (nt + 1) * SQT],
                            start=(dc == 0),
                            stop=(dc == NDC - 1),
                        )
                    nc.vector.tensor_copy(ob[:, nt * SQT : (nt + 1) * SQT], ps_p[:])
                nc.sync.dma_start(out[st_c * P : (st_c + 1) * P, :], ob[:])

        def emit_attention(st, filler):
            """Attention for rep `st`. P.V matmuls are emitted one pass late
            (their exp's latency is covered by the next pass's score matmuls
            -- the PE queue is strictly in-order). Next-rep Q/K filler groups
            are popped at head-pair boundaries, absorbing the norm-chain and
            po-ring latency there and filling the ACT-bound stretches."""
            st.oT = big.tile([P, NDC, s], BF16, tag="oT", name="oT")
            qT, kT, oT, v_sb = st.qT, st.kT, st.oT, st.v_sb
            for sqt in range(n_sqt):
                sq0 = sqt * SQT
                nsk = (sq0 + SQT) // P
                for hp in range(HPC // 2):
                    po = opsum.tile([P, 2, SQT], F32, tag="po", name="po")
                    pv_pending = None

                    def emit_pv(skc, off):
                        for hi in range(2):
                            nc.tensor.matmul(
                                po[0 : DK + 1, hi, off:SQT],
                                v_sb[skc][:, 2 * hp + hi],
                                et_by_skc[skc][:, hi, off:SQT],
                                start=(skc == 0),
                                stop=(skc == nsk - 1),
                            )

                    et_by_skc = {}
                    for skc in range(nsk):
                        off = max(0, skc * P - sq0)
                        ps_s = psum.tile([P, 2, SQT], F32, tag="score", name="ps_s")
                        for hi in range(2):
                            p0 = hi * DK
                            nc.tensor.matmul(
                                ps_s[:, hi, off:SQT],
                                kT[p0 : p0 + DK, hp, skc * P : (skc + 1) * P],
                                qT[p0 : p0 + DK, hp, sq0 + off : sq0 + SQT],
                                start=True,
                                stop=True,
                            )
                        if pv_pending is not None:
                            emit_pv(*pv_pending)
                        et = expp.tile([P, 2, SQT], BF16, tag="exp")
                        et_by_skc[skc] = et
                        nc.scalar.activation(
                            et[:, :, off:SQT],
                            ps_s[:, :, off:SQT],
                            mybir.ActivationFunctionType.Exp,
                            scale=float(1.0 / np.sqrt(DK)),
                        )
                        if skc * P >= sq0:  # diagonal chunk: mask invalid region
                            nc.vector.tensor_tensor(
                                et[:, :, off : off + P],
                                et[:, :, off : off + P],
                                ca_sb[:, None, :].to_broadcast([P, 2, P]),
                                mybir.AluOpType.mult,
                            )
                        pv_pending = (skc, off)
                    emit_pv(*pv_pending)

                    # normalize: oT[dims, sq] = po[0:64, sq] / po[64, sq].
                    # Single-op drain so the po bank frees ASAP.
                    osc = work.tile([DK + 1, 2, SQT], BF16, tag="osc", name="osc")
                    nc.vector.tensor_copy(osc[:], po[0 : DK + 1])
                    for hi in range(2):
                        rec = work.tile([1, SQT], BF16, tag="rec")
                        with nc.allow_low_precision(reason="softmax recip"):
                            nc.vector.reciprocal(rec[:], osc[DK : DK + 1, hi])
                        rec_bc = work.tile([DK, SQT], BF16, tag="rec_bc")
                        nc.gpsimd.partition_broadcast(rec_bc[:], rec[:])
                        nc.vector.tensor_tensor(
                            oT[hi * DK : (hi + 1) * DK, hp, sq0 : sq0 + SQT],
                            osc[0:DK, hi],
                            rec_bc[:],
                            mybir.AluOpType.mult,
                        )
                    # keep the PE fed across the hp boundary (norm + po ring)
                    for _ in range(2):
                        if filler:
                            filler.pop(0)()
                if sqt >= 1:
                    emit_proj(oT, sqt - 1)
            while filler:
                filler.pop(0)()

        def qk_filler(st):
            return [
                (lambda t_i=t_i, name=name, dc=dc: emit_qk_group(st, t_i, name, dc))
                for t_i in range(n_sqt)
                for name in ("wq", "wk")
                for dc in range(NDC)
            ]

        cur = RepState()
        for u in qk_filler(cur):  # rep 0: no previous attention to hide in
            u()
        pending_proj = None
        for r in range(reps):
            if pending_proj is not None:
                emit_proj(*pending_proj)  # before oT/v rings recycle
                pending_proj = None
            emit_v(cur)
            nxt = RepState() if r + 1 < reps else None
            emit_attention(cur, qk_filler(nxt) if nxt else [])
            pending_proj = (cur.oT, n_sqt - 1)
            cur = nxt

        if pending_proj is not None:
            emit_proj(*pending_proj)

    nc.compile()
    return nc


# ---------------- host side ----------------

_ROPE_PERM = None


def _rope_perm():
    """Per-head de-interleave: even dims first, then odd dims."""
    global _ROPE_PERM
    if _ROPE_PERM is None:
        p = []
        for h in range(HPC):
            base = h * DK
            p += [base + 2 * k for k in range(DK // 2)]
            p += [base + 2 * k + 1 for k in range(DK // 2)]
        _ROPE_PERM = np.array(p)
    return _ROPE_PERM


def _bf16():
    import ml_dtypes

    return ml_dtypes.bfloat16


def _host_tables(token_positions, s):
    pos = np.asarray(token_positions).astype(np.float64)
    freqs = THETA ** (-np.arange(0, DK, 2, dtype=np.float64) / DK)  # [32]
    ang = pos[None, :] * freqs[:, None]  # [32, s]
    cos32 = np.cos(ang)
    sin32 = np.sin(ang)
    # layout [128, s]: per head block of 64: [cos32 (x1 half); cos32 (x2 half)]
    cos_t = np.empty((P, s), np.float32)
    sin_t = np.empty((P, s), np.float32)
    for h in range(2):  # 2 heads per 128-partition chunk
        b0 = h * DK
        cos_t[b0 : b0 + 32] = cos32
        cos_t[b0 + 32 : b0 + 64] = cos32
        sin_t[b0 : b0 + 32] = -sin32  # x1 half: -sin * x2
        sin_t[b0 + 32 : b0 + 64] = sin32  # x2 half: +sin * x1
    return cos_t, sin_t


_NC_CACHE = {}

# test harness hooks (off by default; harness calls kernel() directly)
TRACE = False
LAST = {}


def _get_program(b, s, reps=1):
    key = (b, s, reps)
    if key not in _NC_CACHE:
        _NC_CACHE[key] = build_program(b, s, reps)
    return _NC_CACHE[key]


def prepare_in_maps(x, Wq, Wk, Wv, Wo, token_positions):
    bf16 = _bf16()
    x = np.asarray(x, dtype=np.float32)
    Wq = np.asarray(Wq, dtype=np.float32)
    Wk = np.asarray(Wk, dtype=np.float32)
    Wv = np.asarray(Wv, dtype=np.float32)
    Wo = np.asarray(Wo, dtype=np.float32)
    b, s, _ = x.shape

    # [b, kc, p, s] transposed view of x
    xT = np.ascontiguousarray(x.transpose(0, 2, 1)).astype(bf16).reshape(
        b, NKC, P, s
    )
    cos_t, sin_t = _host_tables(token_positions, s)
    cos_t = cos_t.astype(bf16)
    sin_t = sin_t.astype(bf16)
    causal = np.triu(np.ones((P, P), np.float32)).astype(bf16)  # keep p <= f
    # swap permutation matrix (symmetric): swap(j) = j+-32 within each 64-block
    pm = np.zeros((P, P), np.float32)
    for h in range(2):
        b0 = h * DK
        for k in range(32):
            pm[b0 + k + 32, b0 + k] = 1.0
            pm[b0 + k, b0 + k + 32] = 1.0
    pm = pm.astype(bf16)

    perm = _rope_perm()
    in_maps = []
    for c in range(NCORES):
        bi, hh = c // 2, c % 2
        rows = slice(hh * BLK, (hh + 1) * BLK)
        wq_c = Wq[rows][perm]  # [512, D] rope-permuted rows
        wk_c = Wk[rows][perm]
        wv_c = Wv[rows]
        in_maps.append(
            {
                "xT": xT[bi],
                "wqT": np.ascontiguousarray(wq_c.T).astype(bf16).reshape(NKC, P, BLK),
                "wkT": np.ascontiguousarray(wk_c.T).astype(bf16).reshape(NKC, P, BLK),
                "wvT": np.ascontiguousarray(wv_c.T).astype(bf16).reshape(NKC, P, BLK),
                "woT": np.ascontiguousarray(Wo[:, rows].T)
                .astype(bf16)
                .reshape(NDC, P, D),
                "costab": cos_t,
                "sintab": sin_t,
                "pmswap": pm,
                "causal": causal,
            }
        )

    return in_maps


def kernel(x, Wq, Wk, Wv, Wo, token_positions):
    b, s, _ = np.asarray(x).shape
    nc = _get_program(b, s)
    in_maps = prepare_in_maps(x, Wq, Wk, Wv, Wo, token_positions)
    res = run_bass_kernel_spmd(
        nc, in_maps, core_ids=list(range(NCORES)), trace=TRACE
    )
    LAST["exec_time_ns"] = res.exec_time_ns
    LAST["profile_json"] = res.profile_json
    out = np.empty((b, s, D), np.float32)
    for bi in range(b):
        out[bi] = res.results[2 * bi]["out"].astype(np.float32) + res.results[
            2 * bi + 1
        ]["out"].astype(np.float32)
    return out
